# revision 61
# baseline (speedup 1.0000x reference)
"""Multi-head attention (B=2, T=2048, D=1024, H=16) on 8 TRN2 NeuronCores.

Sharding: tensor-parallel over heads — core c owns heads (2c, 2c+1).
Each core computes its heads' QKV projection (column-sharded), full attention
for those heads, and a row-sharded O-projection partial; the host sums the 8
partials and adds b_o (with W_o @ b_v folded in, since softmax rows sum to 1).

Everything on-device runs in bf16 (inputs, weights, probs, staged partials);
PSUM accumulation is fp32, so the relative error stays ~4e-3.

On-device layout (per batch):
  qkv_T [128, 3, 2048]: m0 = q rows (h0 dims 0-63, h1 dims 64-127), m1 = k,
  m2 = v_T. v_T is PE-transposed into v_sb [128keys, kt, 2*65] with a ones
  column per head, so the AV matmul's row 64 accumulates the softmax
  denominators. scores are computed transposed [keys, queries] so the exp
  needs no transposes; no max subtraction (scores ~ N(0, 0.33) here);
  normalization broadcasts 1/sum across partitions via gpsimd, decoupled
  from the PSUM accumulators by a bf16 spill.

Attention runs in per-kt UNITS over 512-query blocks: one unit = both
heads' scores into ONE [128,1024] sc tile (tile_position packs the two
K=64 matmuls into disjoint PE row groups), one combined [128,1024] exp,
and both heads' AV matmuls. PSUM: 3 sc slots (6 banks) + 2 AV accumulators
(2 banks). The 3-deep sc rotation means a unit's scores wait on the exp
two allocations back, so the ~1.4us exp latency chain never lands on the
in-order PE queue even when filler work borrows a slot.

Schedule: 8 attention ranges (2 batches x 4 query blocks) with QKV pieces /
O-projection tiles / x prefetches placed in per-unit filler slots between
the attention units — heavy pieces spread one-per-4-units, O-proj tiles at
most every other unit (their staging-copy latency must clear the sc
rotation). Range seams are warm-started (the next block's first units'
scores+exp pre-run into the previous range's ACT bubble). DMAs issue from
SP in emission order; producer-dependent DMAs are deferred one filler slot
to avoid head-of-line blocking. The tail normalizes the last block in
128-column strips straight from PSUM, software-pipelined with the final
O-projections, with staging copies split across the idle ACT engine + DVE.
"""

import numpy as np

import concourse.bacc as bacc
import concourse.mybir as mybir
import concourse.tile as tile
from concourse import bass_utils

F32 = mybir.dt.float32
BF16 = mybir.dt.bfloat16

B, T, D, H, DH = 2, 2048, 1024, 16, 64
P = 128
NCORES = 8
HPC = H // NCORES          # heads per core = 2
KT = T // P                # key tiles per batch = 16
QB = 512                   # query block (= one token chunk)
NQB = T // QB              # query blocks per (batch, head) = 4
KD = D // P                # contraction tiles for projections = 8


def build_program():
    nc = bacc.Bacc(
        "TRN2",
        target_bir_lowering=False,
        debug=False,
        enable_asserts=False,
        num_devices=NCORES,
    )
    xT = nc.dram_tensor("xT", [D, B * T], BF16, kind="ExternalInput").ap()
    wqkvT = nc.dram_tensor("wqkvT", [D, 3 * P], BF16, kind="ExternalInput").ap()
    bqk = nc.dram_tensor("bqk", [P, 2], F32, kind="ExternalInput").ap()
    wo = nc.dram_tensor("wo", [P, D], BF16, kind="ExternalInput").ap()
    ident_d = nc.dram_tensor("ident", [P, P], BF16, kind="ExternalInput").ap()
    out = nc.dram_tensor("out", [B * T, D], BF16, kind="ExternalOutput").ap()

    with tile.TileContext(nc) as tc:
        _body(tc, xT, wqkvT, bqk, wo, ident_d, out)
    nc.compile()
    return nc


def _body(tc, xT, wqkvT, bqk, wo, ident_d, out):
    nc = tc.nc
    ctxs = []

    def pool(name, bufs, space="SBUF"):
        cm = tc.tile_pool(name=name, bufs=bufs, space=space)
        p = cm.__enter__()
        ctxs.append(cm)
        return p

    const = pool("const", 1)
    xp = pool("xp", 1)             # x [128,8,512] chunk tiles, 4 rotating
    qkvp = pool("qkvp", 2)
    vp = pool("vp", 2)
    probsp = pool("probsp", 3)     # 2 in-flight heads + 1 warm-started
    ocatp = pool("ocatp", 2)
    outp = pool("outp", 3)
    avsp = pool("avsp", 4)         # AV spills (frees PSUM before norm reads)
    recipp = pool("recipp", 2)
    bcp = pool("bcp", 2)
    ps = pool("ps", 1, space="PSUM")   # tags: sc (3x2 banks), av (2x1 banks)

    def ps_sc(name):
        # one slot = 2 banks; 3 slots so a unit's scores wait on the exp
        # two allocations back even when a filler borrows a slot
        return ps.tile([P, 2 * QB], F32, tag="sc", name=name, bufs=3)

    # ---- constants; x chunk0 tiles are interleaved with w tiles so the
    # first projection matmul can start as soon as (x0, w0) land ----
    w_sb = const.tile([P, KD, 3 * P], BF16, name="w_sb")
    wqkv_r = wqkvT.rearrange("(ko p) m -> ko p m", p=P)
    bqk_sb = const.tile([P, 2], F32, name="bqk_sb")
    wo_sb = const.tile([P, D], BF16, name="wo_sb")
    ident = const.tile([P, P], BF16, name="ident")

    xT_r = xT.rearrange("(ko p) t -> ko p t", p=P)

    # DMAs whose producer runs on a compute engine are deferred by one
    # filler slot so their semaphore wait is satisfied by the time the
    # (single, in-order) SP queue reaches them — otherwise they head-of-line
    # block later prefetches.
    deferred = []

    def flush_deferred():
        while deferred:
            deferred.pop(0)()

    def batch_state(b):
        qkvT = qkvp.tile([P, 3, T], BF16, tag="qkv", name=f"qkv_{b}")
        v_sb = vp.tile([P, KT, 2 * (DH + 1)], BF16, tag="v", name=f"v_{b}")
        v4 = v_sb.rearrange("p t (g c) -> p t g c", g=2)
        nc.vector.memset(v4[:, :, :, DH:DH + 1], 1.0)
        ocat = ocatp.tile([P, T], BF16, tag="ocat", name=f"ocat_{b}")
        return dict(b=b, qkvT=qkvT, v4=v4, v_sb=v_sb, ocat=ocat,
                    attn={}, xc={})

    def prefetch_x(st, n, bundle=4):
        """Issue chunk n's x DMAs: one [128, bundle, 512] transfer per
        `bundle` k-tiles (fewer HWDGE slots; startup uses bundle=1)."""
        b = st["b"]
        if n in st["xc"]:
            return
        xc = xp.tile([P, KD, 512], BF16, tag="x", name=f"x_{b}_{n}", bufs=4)
        src = xT_r[:, :, b * T + n * 512: b * T + (n + 1) * 512]
        src = src.rearrange("ko p t -> p ko t")
        for j in range(0, KD, bundle):
            nc.sync.dma_start(xc[:, j:j + bundle, :], src[:, j:j + bundle, :])
        st["xc"][n] = [xc[:, k, :] for k in range(KD)]

    def emit_qkv_piece(st, n, m, half=None):
        """One projection piece: qkv output block m for 512-token chunk n.
        Borrows one sc slot (8 accumulating matmuls, then DVE drains it).
        half=0/1 emits only the first/second 4 k-tiles (split filler)."""
        b, qkvT = st["b"], st["qkvT"]
        prefetch_x(st, n)
        xc = st["xc"][n]
        if half in (None, 0):
            st["pq"] = ps_sc(f"qkvps_{b}_{m}_{n}")
        pq = st["pq"]
        ks = range(KD) if half is None else (
            range(KD // 2) if half == 0 else range(KD // 2, KD))
        for k in ks:
            nc.tensor.matmul(
                pq[:, :512],
                w_sb[:, k, m * P:(m + 1) * P],
                xc[k],
                start=(k == 0),
                stop=(k == KD - 1),
            )
        if half == 0:
            return
        dst = qkvT[:, m, n * 512:(n + 1) * 512]
        if m < 2:
            nc.vector.tensor_scalar_add(dst, pq[:, :512], bqk_sb[:, m:m + 1])
        else:
            nc.vector.tensor_copy(out=dst, in_=pq[:, :512])

    def emit_vt(st, n):
        """Transpose chunk n of v (qkvT m=2) into v_sb via PE transpose
        (borrowing one sc slot) + DVE copy, as 4 key tiles of [128,128]."""
        b, qkvT, v4 = st["b"], st["qkvT"], st["v4"]
        pv = ps_sc(f"vt_{b}_{n}").bitcast(BF16)[:, :512]
        for j in range(4):
            tt = 4 * n + j
            nc.tensor.transpose(pv[:, j * P:(j + 1) * P],
                                qkvT[:, 2, tt * P:(tt + 1) * P], ident)
        nc.vector.tensor_copy(
            out=v4[:, 4 * n:4 * n + 4, :, 0:DH],
            in_=pv.rearrange("p (t g c) -> p t g c", t=4, g=2),
        )

    def emit_oproj_tt(st, tt, on_act=False):
        """One O-projection token tile: 2 matmuls into a borrowed sc slot,
        copy to a bf16 staging tile (DVE, or ACT at the tail when the
        activation engine is idle), then DMA the partial to DRAM."""
        b, ocat = st["b"], st["ocat"]
        po = ps_sc(f"op_{b}_{tt}")
        for nn in range(D // 512):
            nc.tensor.matmul(
                po[:, nn * 512:(nn + 1) * 512],
                ocat[:, tt * P:(tt + 1) * P],
                wo_sb[:, nn * 512:(nn + 1) * 512],
                start=True,
                stop=True,
            )
        ob = outp.tile([P, D], BF16, tag="ob", name=f"ob_{b}_{tt}")
        if on_act:
            nc.scalar.activation(ob, po, mybir.ActivationFunctionType.Copy)
        else:
            nc.vector.tensor_copy(out=ob, in_=po)
        deferred.append(lambda: nc.sync.dma_start(
            out[b * T + tt * P: b * T + (tt + 1) * P, :], ob))

    # Attention is organized in per-kt UNITS. One unit = both heads'
    # scores for this query block into ONE sc tile (h0 in cols 0-511, h1 in
    # 512-1023), one combined exp, and both heads' AV matmuls. With one sc
    # alloc per unit and a 3-deep rotation, a unit's scores wait on the exp
    # two units back — the exp latency chain (sem + 1us exp + pipeline) is
    # fully hidden behind PE work even when fillers borrow a slot.

    def attn_state(st, qb):
        if qb not in st["attn"]:
            b = st["b"]
            st["attn"][qb] = dict(
                # [keys-part, kt, head, 512 queries]
                probs=probsp.tile([P, KT, HPC, QB], BF16, tag="probs",
                                  name=f"pb_{b}_{qb}", bufs=3),
                av=[ps.tile([DH + 1, QB], F32, tag="av",
                            name=f"av_{b}_{qb}_{h}", bufs=2)
                    for h in range(HPC)],
                warm=0)
        return st["attn"][qb]

    def emit_unit_scores(st, qb, kt):
        b, qkvT = st["b"], st["qkvT"]
        q0 = qb * QB
        s = ps_sc(f"s_{b}_{qb}_{kt}")
        for h in range(HPC):
            hs = h * DH
            nc.tensor.matmul(
                s[:, h * QB:(h + 1) * QB],
                qkvT[hs:hs + DH, 1, kt * P:(kt + 1) * P],
                qkvT[hs:hs + DH, 0, q0:q0 + QB],
                start=True,
                stop=True,
                tile_position=(hs, 0),
            )
        return s

    def emit_unit_exp(st, qb, kt, s):
        a = st["attn"][qb]
        nc.scalar.activation(
            a["probs"][:, kt, :, :], s,
            mybir.ActivationFunctionType.Exp,
        )

    def emit_unit_av(st, qb, kt):
        a = st["attn"][qb]
        v4 = st["v4"]
        for h in range(HPC):
            nc.tensor.matmul(
                a["av"][h][:, :],
                v4[:, kt, h, :],  # [128, 65]
                a["probs"][:, kt, h, :],
                start=(kt == 0),
                stop=(kt == KT - 1),
            )

    def emit_warm(st, qb, k=1):
        """Pre-run scores+exp for the NEXT query block's first k units, so
        its first AVs never wait on the activation engine at the range seam
        (deeper warms shift exp work into a PE-bound range's idle ACT)."""
        a = attn_state(st, qb)
        for u in range(a["warm"], k):
            emit_unit_exp(st, qb, u, emit_unit_scores(st, qb, u))
        a["warm"] = k

    def emit_attn_range(st, qb, fillers=()):
        """All 16 attention units for query block qb. One filler piece per
        unit (None entries = no filler); deferred DMAs flush one slot later.
        scores(kt+1) is emitted BEFORE av(kt) so the next exp's input never
        queues behind AV matmuls on the in-order PE."""
        fillers = list(fillers)
        a = attn_state(st, qb)
        if not a["warm"]:
            emit_unit_exp(st, qb, 0, emit_unit_scores(st, qb, 0))
            a["warm"] = 1
        for kt in range(KT):
            emit_nxt = kt + 1 < KT and kt + 1 >= a["warm"]
            if emit_nxt:
                s_nxt = emit_unit_scores(st, qb, kt + 1)
            flush_deferred()
            if fillers:
                f = fillers.pop(0)
                if f is not None:
                    f()
            emit_unit_av(st, qb, kt)
            if emit_nxt:
                emit_unit_exp(st, qb, kt + 1, s_nxt)
        for f in fillers:   # leftovers (shouldn't happen if slots >= fillers)
            if f is not None:
                f()
        flush_deferred()

    def emit_spill(st, qb):
        """Copy the finished AV accumulators PSUM->SBUF (bf16) so the av
        PSUM banks free after one copy instead of after the whole
        reciprocal/broadcast/multiply normalization chain."""
        b = st["b"]
        a = st["attn"][qb]
        a["avs"] = []
        for h in range(HPC):
            avs = avsp.tile([DH + 1, QB], BF16, tag="avs",
                            name=f"avs_{b}_{qb}_{h}")
            nc.vector.tensor_copy(out=avs, in_=a["av"][h])
            a["avs"].append(avs)

    def emit_norm(st, qb, from_psum=False):
        b, ocat = st["b"], st["ocat"]
        q0 = qb * QB
        avs = st["attn"][qb]["av" if from_psum else "avs"]
        for h in range(HPC):
            hs = h * DH
            recip = recipp.tile([1, QB], F32, tag="recip",
                                name=f"rc_{b}_{qb}_{h}")
            nc.vector.reciprocal(recip, avs[h][DH:DH + 1, :])
            bc = bcp.tile([DH, QB], F32, tag="bc", name=f"bc_{b}_{qb}_{h}")
            nc.gpsimd.partition_broadcast(bc, recip)
            nc.vector.tensor_mul(
                out=ocat[hs:hs + DH, q0:q0 + QB],
                in0=avs[h][0:DH, :], in1=bc)

    # ---- schedule ----
    def J(*fs):
        """join several emissions into one filler slot"""
        def g():
            for f in fs:
                f()
        return g

    def QK(st, n, m):
        return lambda: emit_qkv_piece(st, n, m)

    def QKV(st, n):
        """chunk n's v piece + its deferred transpose"""
        return J(QK(st, n, 2), lambda: emit_vt(st, n))

    def OP(st, tt):
        return lambda: emit_oproj_tt(st, tt)

    def PF(st, n):
        return lambda: prefetch_x(st, n)

    def QH(st, n, m, half):
        return lambda: emit_qkv_piece(st, n, m, half=half)

    s0 = batch_state(0)
    s1 = batch_state(1)

    # startup: bundled DMAs (HWDGE overhead dominates small transfers); the
    # q-block of w goes first so the first projection can start early. SP
    # carries the startup-critical w + x; ACT's queue carries the rest.
    wr = wqkv_r.rearrange("ko p m -> p ko m")
    nc.sync.dma_start(ident, ident_d)
    nc.sync.dma_start(w_sb[:, 0:KD // 2, :], wr[:, 0:KD // 2, :])
    prefetch_x(s0, 0, bundle=2)
    nc.sync.dma_start(w_sb[:, KD // 2:, :], wr[:, KD // 2:, :])
    prefetch_x(s0, 1, bundle=4)
    nc.scalar.dma_start(bqk_sb, bqk)
    nc.scalar.dma_start(wo_sb, wo)
    # PE pstate warm-up on ident (tiny DMA, lands in ~1.5us)
    pwu = ps_sc("pwu")
    for _ in range(3):
        nc.tensor.transpose(pwu.bitcast(BF16)[:, 0:P], ident, ident)
    # pre-attention: q,k,v of chunk 0, then warm (b0, qb0)
    emit_qkv_piece(s0, 0, 0)
    emit_qkv_piece(s0, 0, 1)
    emit_qkv_piece(s0, 0, 2)
    emit_warm(s0, 0, 2)
    emit_qkv_piece(s0, 1, 1)
    emit_vt(s0, 0)

    def rng(st, qb, fillers, warm=None, last=False):
        fillers = list(fillers)
        assert len(fillers) <= (16 if warm is None else 15)
        fillers += [None] * (15 - len(fillers))
        if len(fillers) == 15:
            fillers.append(warm)
        emit_attn_range(st, qb, fillers=fillers)
        if not last:
            emit_spill(st, qb)
            emit_norm(st, qb)

    # R(0,0): batch0's remaining k/v pieces (scores kt4+ need ch1 m1, kt8+
    # ch2, kt12+ ch3; av kt4/8/12 need the matching v transposed) + ch1 q
    rng(s0, 0, [
        QKV(s0, 1),
        None,
        J(PF(s0, 2), QK(s0, 2, 1)),
        QKV(s0, 2),
        J(PF(s0, 3), QK(s0, 3, 1)),
        QKV(s0, 3),
        QK(s0, 1, 0),
    ], warm=lambda: emit_warm(s0, 1, 2))
    # mid ranges: batch1 projections spread evenly (one heavy piece per 4
    # units keeps the PE backlog covering the activation-paced stretches);
    # O-proj fillers alternate with empty units so a scores allocation three
    # slots back never lands on an undrained staging copy
    rng(s0, 1, [
        J(PF(s1, 0), QK(s1, 0, 0)), None, None, None,
        QK(s1, 0, 1), None, None, None,
        QKV(s1, 0), None, None, None,
        QK(s0, 2, 0), OP(s0, 0),
    ], warm=lambda: emit_warm(s0, 2, 2))
    rng(s0, 2, [
        J(PF(s1, 1), QK(s1, 1, 0)), None, None, None,
        QK(s1, 1, 1), None, None, None,
        QKV(s1, 1), None, None, None,
        QK(s0, 3, 0), OP(s0, 1),
    ], warm=lambda: emit_warm(s0, 3, 2))
    rng(s0, 3, [
        J(PF(s1, 2), QK(s1, 2, 0)), None, OP(s0, 2), None,
        QK(s1, 2, 1), None, OP(s0, 3), None,
        QKV(s1, 2), None, None, None,
        None, OP(s0, 4),
    ], warm=lambda: emit_warm(s1, 0, 2))
    rng(s1, 0, [
        J(PF(s1, 3), QK(s1, 3, 0)), None, OP(s0, 5), None,
        QK(s1, 3, 1), None, OP(s0, 6), None,
        QKV(s1, 3), None, OP(s0, 7), None,
        None, OP(s0, 8),
    ], warm=lambda: emit_warm(s1, 1, 2))
    rng(s1, 1, [
        OP(s0, 9), None, OP(s0, 10), None,
        OP(s0, 11), None, OP(s0, 12), None,
        OP(s0, 13), None, OP(s0, 14), None,
        OP(s0, 15), None, OP(s1, 0),
    ], warm=lambda: emit_warm(s1, 2, 2))
    rng(s1, 2, [
        None, None, None, None,
        OP(s1, 1), None, OP(s1, 2), None,
        OP(s1, 3), None, OP(s1, 4), None,
        OP(s1, 5), None, OP(s1, 6),
    ], warm=lambda: emit_warm(s1, 3, 2))
    rng(s1, 3, [
        None, None, None, None, None,
        OP(s1, 7), None, OP(s1, 8), None,
        OP(s1, 9), None, OP(s1, 10), None,
        None, None, OP(s1, 11),
    ], last=True)
    # tail: normalize the last block straight from PSUM (no one needs the
    # banks), O-proj the last 4 tiles with each staging copy split between
    # the (now idle) activation engine and DVE
    # tail: normalize the last block in 128-column strips straight from
    # PSUM, launching each token tile's O-projection as soon as its strip is
    # ready; staging copies split between the (now idle) activation engine
    # and DVE
    av = s1["attn"][3]["av"]
    ocat = s1["ocat"]
    TT = (12, 13, 14, 15)
    recips, bcs = {}, {}

    def strip_recip(tt):
        c0 = tt * P - 3 * QB
        for h in range(HPC):
            r = recipp.tile([1, P], F32, tag="recip_t",
                            name=f"rc_t_{tt}_{h}", bufs=8)
            nc.vector.reciprocal(r, av[h][DH:DH + 1, c0:c0 + P])
            recips[tt, h] = r
        for h in range(HPC):
            bc = bcp.tile([DH, P], F32, tag="bc_t", name=f"bc_t_{tt}_{h}",
                          bufs=8)
            nc.gpsimd.partition_broadcast(bc, recips[tt, h])
            bcs[tt, h] = bc

    def strip_out(tt):
        c0 = tt * P - 3 * QB
        for h in range(HPC):
            hs = h * DH
            nc.vector.tensor_mul(
                out=ocat[hs:hs + DH, tt * P:(tt + 1) * P],
                in0=av[h][0:DH, c0:c0 + P], in1=bcs[tt, h])
        po = ps_sc(f"op_1_{tt}")
        for nn in range(D // 512):
            nc.tensor.matmul(
                po[:, nn * 512:(nn + 1) * 512],
                ocat[:, tt * P:(tt + 1) * P],
                wo_sb[:, nn * 512:(nn + 1) * 512],
                start=True,
                stop=True,
            )
        ob = outp.tile([P, D], BF16, tag="ob", name=f"ob_1_{tt}")
        nc.scalar.activation(ob[:, 0:640], po[:, 0:640],
                             mybir.ActivationFunctionType.Copy)
        nc.vector.tensor_copy(out=ob[:, 640:], in_=po[:, 640:])
        nc.sync.dma_start(out[T + tt * P: T + (tt + 1) * P, 0:640],
                          ob[:, 0:640])
        nc.sync.dma_start(out[T + tt * P: T + (tt + 1) * P, 640:],
                          ob[:, 640:])

    strip_recip(12)
    strip_recip(13)
    strip_out(12)
    strip_recip(14)
    strip_out(13)
    strip_recip(15)
    strip_out(14)
    strip_out(15)

    for cm in reversed(ctxs):
        cm.__exit__(None, None, None)


def _bf16_np():
    import ml_dtypes
    return ml_dtypes.bfloat16


def host_inputs(x, W_qkv, b_qkv, W_o, b_o):
    """Per-core input dicts."""
    bf16 = _bf16_np()
    x = np.asarray(x, dtype=np.float32)
    W_qkv = np.asarray(W_qkv, dtype=np.float32)
    b_qkv = np.asarray(b_qkv, dtype=np.float32)
    W_o = np.asarray(W_o, dtype=np.float32)

    xT = np.ascontiguousarray(x.reshape(B * T, D).T).astype(bf16)
    scale = DH ** -0.5
    in_maps = []
    for c in range(NCORES):
        heads = [HPC * c + i for i in range(HPC)]
        cols = []
        biases_qk = []
        for blk, sc in ((0, scale), (1, 1.0)):  # q, k
            for h in heads:
                r = blk * D + h * DH
                cols.append(W_qkv[r:r + DH].T * sc)
                biases_qk.append(b_qkv[r:r + DH] * sc)
        for h in heads:                          # v
            r = 2 * D + h * DH
            cols.append(W_qkv[r:r + DH].T)
        wqkvT = np.ascontiguousarray(np.concatenate(cols, axis=1)).astype(bf16)
        bqk = np.ascontiguousarray(
            np.stack([np.concatenate(biases_qk[:HPC]),
                      np.concatenate(biases_qk[HPC:])], axis=1))
        wo = np.ascontiguousarray(
            np.concatenate([W_o[:, h * DH:(h + 1) * DH] for h in heads],
                           axis=1).T).astype(bf16)
        in_maps.append({"xT": xT, "wqkvT": wqkvT, "bqk": bqk, "wo": wo,
                        "ident": np.eye(P).astype(bf16)})
    return in_maps


_NC_CACHE = {}


def get_nc():
    if "nc" not in _NC_CACHE:
        _NC_CACHE["nc"] = build_program()
    return _NC_CACHE["nc"]


def kernel(x, W_qkv, b_qkv, W_o, b_o, _results=None):
    in_maps = host_inputs(x, W_qkv, b_qkv, W_o, b_o)
    if _results is None:
        res = bass_utils.run_bass_kernel_spmd(
            get_nc(), in_maps, core_ids=list(range(NCORES)))
        _results = res.results
    acc = _results[0]["out"].astype(np.float32)
    for c in range(1, NCORES):
        acc = acc + _results[c]["out"]
    W_o = np.asarray(W_o, np.float32)
    b_qkv = np.asarray(b_qkv, np.float32)
    bias = np.asarray(b_o, np.float32) + W_o @ b_qkv[2 * D:3 * D]
    acc = acc + bias
    return acc.reshape(B, T, D)
